# revision 1
# baseline (speedup 1.0000x reference)
"""Trainium2 Bass kernel for cosine-similarity contrastive loss (CosSimLoss).

reference:
    p = l2norm(pred).reshape(-1, C); t = l2norm(target).reshape(-1, C)
    logits = (p @ t.T) * e^0.5
    loss = mean(logsumexp(logits, axis=1) - diag(logits))

Strategy (8 NeuronCores, data parallel over the N = B*L = 8192 row dim):
  Each core gets a 1024-row shard of pred plus the full target. On-device it
  normalizes its pred shard and the full target (work duplicated across
  cores - cheap next to the N/8 x N matmul), casts to bf16, transposes both
  via the DMA xbar (DRAM bounce) to get contraction-on-partitions layouts,
  computes its 1024 x 8192 logits tile with TensorE (bf16, fp32 PSUM accum),
  applies exp on ScalarE fused with the e^0.5 scale and row-sum accumulation,
  and emits per-row (logsumexp - diag) partials. Since |cos|<=1, exp never
  overflows, so no max-subtraction pass is needed. The diagonal is computed
  exactly (fp32) from the pred shard and the matching target rows (passed as
  a separate pre-sliced input so the program stays SPMD). Host sums the
  per-core partials and divides by N.
"""
import math

import numpy as np

import concourse.bacc as bacc
import concourse.mybir as mybir
import concourse.tile as tile
from concourse.bass_utils import run_bass_kernel_spmd

F32 = mybir.dt.float32
BF16 = mybir.dt.bfloat16
AF = mybir.ActivationFunctionType
ALU = mybir.AluOpType
AXIS = mybir.AxisListType

TEMPERATURE = 0.5
SCALE = float(math.exp(TEMPERATURE))

# Full problem config (hardcoded per contest rules).
B, L, C = 4, 2048, 512
N_CORES = 8
N_TOTAL = B * L                  # 8192
M_LOCAL = N_TOTAL // N_CORES     # 1024 rows per core


def build_nc(m_local=M_LOCAL, n=N_TOTAL, c=C, blk=2048):
    """Build + compile the per-core Bass program (SPMD: same NEFF, 8 cores)."""
    assert c % 128 == 0 and n % 128 == 0 and m_local % 128 == 0
    blk = min(blk, n)
    assert n % blk == 0 and blk % 512 == 0
    kt = c // 128                   # contraction chunks
    mt = m_local // 128             # output row tiles
    nb = n // blk                   # n blocks (psum groups)
    jt = blk // 512                 # 512-wide psum slices per block
    qrows = 512                     # rows per staging quad

    nc = bacc.Bacc("TRN2", target_bir_lowering=False, debug=False)
    pred = nc.dram_tensor("pred", [m_local, c], F32, kind="ExternalInput").ap()
    tgt = nc.dram_tensor("tgt", [n, c], F32, kind="ExternalInput").ap()
    td = nc.dram_tensor("td", [m_local, c], F32, kind="ExternalInput").ap()
    out = nc.dram_tensor("out", [128, mt], F32, kind="ExternalOutput").ap()

    def quad_dram_ap(t2d, r0, rows):
        # DRAM rows [r0, r0+rows) viewed as [128 part, rows//128, c]
        return t2d[r0:r0 + rows, :].rearrange("(q p) c -> p q c", p=128)

    with tile.TileContext(nc) as tc:
        with (
            tc.tile_pool(name="dram", bufs=2, space="DRAM") as dram_pool,
            tc.tile_pool(name="pq", bufs=1) as p_pool,
            tc.tile_pool(name="tdq", bufs=1) as td_pool,
            tc.tile_pool(name="tq", bufs=6) as t_pool,
            tc.tile_pool(name="tbq", bufs=3) as tb_pool,
            tc.tile_pool(name="sq", bufs=2) as sq_pool,
            tc.tile_pool(name="stats", bufs=2) as stats_pool,
            tc.tile_pool(name="pT", bufs=1) as pT_pool,
            tc.tile_pool(name="tT", bufs=2) as tT_pool,
            tc.tile_pool(name="expsc", bufs=2) as exp_pool,
            tc.tile_pool(name="psum", bufs=2, space="PSUM") as psum_pool,
        ):
            # ---------------- Phase A: pred shard + exact diagonal ----------
            pquads = []           # (tile, n_slices)
            sp = stats_pool.tile([128, mt], F32)
            st = stats_pool.tile([128, mt], F32)
            d0 = stats_pool.tile([128, mt], F32)
            for r0 in range(0, m_local, qrows):
                rows = min(qrows, m_local - r0)
                ns = rows // 128
                pq = p_pool.tile([128, (qrows // 128) * c], F32, name=f"pq{r0}")
                nc.sync.dma_start(pq[:, :ns * c].rearrange("p (q c) -> p q c", c=c),
                                  quad_dram_ap(pred, r0, rows))
                tdq = td_pool.tile([128, (qrows // 128) * c], F32, name=f"tdq{r0}")
                nc.sync.dma_start(tdq[:, :ns * c].rearrange("p (q c) -> p q c", c=c),
                                  quad_dram_ap(td, r0, rows))
                m0 = r0 // 128
                for s in range(ns):
                    a = pq[:, s * c:(s + 1) * c]
                    b = tdq[:, s * c:(s + 1) * c]
                    sq1 = sq_pool.tile([128, c], F32, name="sqd")
                    nc.vector.scalar_tensor_tensor(
                        sq1[:], a, 1.0, a, ALU.mult, ALU.mult,
                        accum_out=sp[:, m0 + s:m0 + s + 1])
                    sq2 = sq_pool.tile([128, c], F32, name="sqd")
                    nc.vector.scalar_tensor_tensor(
                        sq2[:], b, 1.0, b, ALU.mult, ALU.mult,
                        accum_out=st[:, m0 + s:m0 + s + 1])
                    sq3 = sq_pool.tile([128, c], F32, name="sqd")
                    nc.vector.scalar_tensor_tensor(
                        sq3[:], a, 1.0, b, ALU.mult, ALU.mult,
                        accum_out=d0[:, m0 + s:m0 + s + 1])
                pquads.append((pq, ns))

            # rp = sp^-0.5 = exp(-0.5 ln sp); rtd likewise (no table switches:
            # ln/exp/square/copy all live in natural_log_exp_and_others).
            lp = stats_pool.tile([128, mt], F32)
            rp = stats_pool.tile([128, mt], F32)
            nc.scalar.activation(lp[:], sp[:], AF.Ln)
            nc.scalar.activation(rp[:], lp[:], AF.Exp, scale=-0.5)
            lt = stats_pool.tile([128, mt], F32)
            rtd = stats_pool.tile([128, mt], F32)
            nc.scalar.activation(lt[:], st[:], AF.Ln)
            nc.scalar.activation(rtd[:], lt[:], AF.Exp, scale=-0.5)

            # diag_scaled = d0 * rp * rtd * SCALE
            dtmp = stats_pool.tile([128, mt], F32)
            nc.vector.tensor_mul(dtmp[:], d0[:], rp[:])
            diag = stats_pool.tile([128, mt], F32)
            nc.vector.scalar_tensor_tensor(
                diag[:], dtmp[:], SCALE, rtd[:], ALU.mult, ALU.mult)

            # normalized bf16 pred -> DRAM bounce -> xbar transpose -> pT
            p_bf = dram_pool.tile([m_local, c], BF16)
            for qi, (pq, ns) in enumerate(pquads):
                r0 = qi * qrows
                m0 = r0 // 128
                pbq = tb_pool.tile([128, (qrows // 128) * c], BF16, name="pbq")
                for s in range(ns):
                    nc.vector.tensor_scalar_mul(
                        pbq[:, s * c:(s + 1) * c], pq[:, s * c:(s + 1) * c],
                        rp[:, m0 + s:m0 + s + 1])
                nc.sync.dma_start(quad_dram_ap(p_bf, r0, ns * 128),
                                  pbq[:, :ns * c].rearrange("p (q c) -> p q c", c=c))
            pT = []
            for k in range(kt):
                pTk = pT_pool.tile([128, m_local], BF16, name=f"pT{k}")
                nc.sync.dma_start_transpose(
                    pTk[:], p_bf[0:m_local, k * 128:(k + 1) * 128])
                pT.append(pTk)

            # ---------------- Phase B: target blocks + matmul + exp ---------
            sume = stats_pool.tile([128, mt * nb], F32)
            for g in range(nb):
                gr0 = g * blk
                nq = blk // qrows
                stt = stats_pool.tile([128, blk // 128], F32, name="stt")
                tquads = []
                for qi in range(nq):
                    r0 = gr0 + qi * qrows
                    tq = t_pool.tile([128, (qrows // 128) * c], F32, name="tq")
                    nc.sync.dma_start(tq[:].rearrange("p (q c) -> p q c", c=c),
                                      quad_dram_ap(tgt, r0, qrows))
                    for s in range(4):
                        a = tq[:, s * c:(s + 1) * c]
                        sq4 = sq_pool.tile([128, c], F32, name="sqd")
                        nc.vector.scalar_tensor_tensor(
                            sq4[:], a, 1.0, a, ALU.mult, ALU.mult,
                            accum_out=stt[:, qi * 4 + s:qi * 4 + s + 1])
                    tquads.append(tq)
                ltt = stats_pool.tile([128, blk // 128], F32, name="ltt")
                rtt = stats_pool.tile([128, blk // 128], F32, name="rtt")
                nc.scalar.activation(ltt[:], stt[:], AF.Ln)
                nc.scalar.activation(rtt[:], ltt[:], AF.Exp, scale=-0.5)

                t_bf = dram_pool.tile([blk, c], BF16, name="t_bf")
                for qi in range(nq):
                    r0 = qi * qrows
                    tbq = tb_pool.tile([128, (qrows // 128) * c], BF16, name="tbq")
                    for s in range(4):
                        nc.vector.tensor_scalar_mul(
                            tbq[:, s * c:(s + 1) * c],
                            tquads[qi][:, s * c:(s + 1) * c],
                            rtt[:, qi * 4 + s:qi * 4 + s + 1])
                    nc.sync.dma_start(quad_dram_ap(t_bf, r0, qrows),
                                      tbq[:].rearrange("p (q c) -> p q c", c=c))
                tT = []
                for k in range(kt):
                    tTk = tT_pool.tile([128, blk], BF16, name=f"tT{k}")
                    nc.sync.dma_start_transpose(
                        tTk[:], t_bf[0:blk, k * 128:(k + 1) * 128])
                    tT.append(tTk)

                for m in range(mt):
                    ps = psum_pool.tile([128, blk], F32, name="ps")
                    for k in range(kt):
                        for j in range(jt):
                            nc.tensor.matmul(
                                ps[:, j * 512:(j + 1) * 512],
                                pT[k][:, m * 128:(m + 1) * 128],
                                tT[k][:, j * 512:(j + 1) * 512],
                                start=(k == 0), stop=(k == kt - 1))
                    esc = exp_pool.tile([128, blk], BF16, name="esc")
                    nc.scalar.activation(
                        esc[:], ps[:], AF.Exp, scale=SCALE,
                        accum_out=sume[:, m * nb + g:m * nb + g + 1])

            # ---------------- Phase C: lse - diag ---------------------------
            if nb > 1:
                rowsum = stats_pool.tile([128, mt], F32)
                nc.vector.tensor_reduce(
                    rowsum[:], sume[:].rearrange("p (m g) -> p m g", g=nb),
                    axis=AXIS.X, op=ALU.add)
            else:
                rowsum = sume
            lse = stats_pool.tile([128, mt], F32)
            nc.scalar.activation(lse[:], rowsum[:], AF.Ln)
            losst = stats_pool.tile([128, mt], F32)
            nc.vector.tensor_sub(losst[:], lse[:], diag[:])
            nc.sync.dma_start(out[:], losst[:])

    nc.compile()
    return nc


_NC_CACHE = {}


def _get_nc():
    key = (M_LOCAL, N_TOTAL, C)
    if key not in _NC_CACHE:
        _NC_CACHE[key] = build_nc()
    return _NC_CACHE[key]


def run_cores(pred2d, tgt2d, trace=False):
    """Run the SPMD program on cores 0..7; returns (partials [8,128,mt], res)."""
    nc = _get_nc()
    in_maps = []
    for ci in range(N_CORES):
        r0 = ci * M_LOCAL
        in_maps.append({
            "pred": np.ascontiguousarray(pred2d[r0:r0 + M_LOCAL]),
            "tgt": np.ascontiguousarray(tgt2d),
            "td": np.ascontiguousarray(tgt2d[r0:r0 + M_LOCAL]),
        })
    res = run_bass_kernel_spmd(nc, in_maps, list(range(N_CORES)), trace=trace)
    partials = np.stack([res.results[i]["out"] for i in range(N_CORES)])
    return partials, res


def kernel(pred, target):
    pred2d = np.asarray(pred, dtype=np.float32).reshape(-1, C)
    tgt2d = np.asarray(target, dtype=np.float32).reshape(-1, C)
    partials, _ = run_cores(pred2d, tgt2d)
    loss = partials.astype(np.float64).sum() / float(N_TOTAL)
    return np.float32(loss)

